# revision 40
# baseline (speedup 1.0000x reference)
"""Trainium2 Bass kernel for an 8-batch image-conditioned decoder layer.

Strategy: pure data-parallel over the batch — core c computes batch element c
end-to-end (embedding gather, causal self-attention, cross-attention over the
image tokens, both layernorms, vocab projection). No collectives.

v6 = v1 baseline schedule plus: exact 32000-wide vocab projection (62 full
chunks + one 256 chunk, no padding), LN2 affine folded into Wp/bp on the host,
in-place PSUM causal masking, and K2/V2 warmup first with weight DMAs
staggered (pool-slot sequencing) so the early HBM window serves the
latency-critical gather/pos/img loads.
"""

import os
import sys

for _p in ("/opt/trn_rl_repo", "/root/.axon_site/_ro/trn_rl_repo"):
    if os.path.isdir(_p) and _p not in sys.path:
        sys.path.append(_p)

import numpy as np
import ml_dtypes

BF16 = ml_dtypes.bfloat16

# Problem dims (hardcoded per spec)
V, D, DI, S, B, NI = 32000, 1024, 768, 512, 8, 197
EPS = 1e-5
P = 128
ST = S // P          # 4 seq tiles
DT = D // P          # 8 model-dim tiles
DIT = DI // P        # 6 image-dim tiles
NIT = 2              # image tokens: 197 -> 2 partition tiles (128 + 69)
NI2 = NI - P         # 69
CN = 512             # vocab chunk width
NFULL = V // CN      # 62 full chunks
CLAST = V - NFULL * CN   # 256
GRP = 2              # chunks per output strip
NGRP = NFULL // GRP  # 31
N_CORES = 8
SCALE = 1.0 / float(np.sqrt(np.float32(D)))

_CACHE = {}
LAST_RESULTS = None


def _build_program():
    import concourse.bacc as bacc
    import concourse.bass as bass
    import concourse.mybir as mybir
    from concourse.masks import make_identity
    from concourse.tile import TileContext

    f32 = mybir.dt.float32
    bf16 = mybir.dt.bfloat16
    i32 = mybir.dt.int32
    X = mybir.AxisListType.X
    ALU = mybir.AluOpType
    ACT_F = mybir.ActivationFunctionType

    nc = bacc.Bacc("TRN2", target_bir_lowering=False, debug=False,
                   num_devices=N_CORES)

    # ---- I/O ----
    h_tok = nc.dram_tensor("tok", [S], i32, kind="ExternalInput")
    h_table = nc.dram_tensor("table", [V, D], bf16, kind="ExternalInput")
    h_pos = nc.dram_tensor("pos", [S, D], bf16, kind="ExternalInput")
    h_img = nc.dram_tensor("img_t", [P, DIT, NI], bf16, kind="ExternalInput")
    h_wq1 = nc.dram_tensor("wq1", [P, DT, D], bf16, kind="ExternalInput")
    h_wk1 = nc.dram_tensor("wk1", [P, DT, D], bf16, kind="ExternalInput")
    h_wv1 = nc.dram_tensor("wv1", [P, DT, D], bf16, kind="ExternalInput")
    h_wq2 = nc.dram_tensor("wq2", [P, DT, D], bf16, kind="ExternalInput")
    h_wk2 = nc.dram_tensor("wk2", [P, DIT, D], bf16, kind="ExternalInput")
    h_wv2 = nc.dram_tensor("wv2", [P, DIT, D], bf16, kind="ExternalInput")
    h_wp = nc.dram_tensor("wp", [NFULL, P, DT, CN], bf16, kind="ExternalInput")
    h_wpl = nc.dram_tensor("wpl", [P, DT, CLAST], bf16, kind="ExternalInput")
    h_bq1 = nc.dram_tensor("bq1", [P, DT], f32, kind="ExternalInput")
    h_bk1 = nc.dram_tensor("bk1", [P, DT], f32, kind="ExternalInput")
    h_bq2 = nc.dram_tensor("bq2", [P, DT], f32, kind="ExternalInput")
    h_bk2 = nc.dram_tensor("bk2", [P, DT], f32, kind="ExternalInput")
    h_bv1 = nc.dram_tensor("bv1", [D], f32, kind="ExternalInput")
    h_bv2 = nc.dram_tensor("bv2", [D], f32, kind="ExternalInput")
    h_bp = nc.dram_tensor("bp", [V], bf16, kind="ExternalInput")
    h_g1 = nc.dram_tensor("g1", [D], f32, kind="ExternalInput")
    h_b1 = nc.dram_tensor("b1", [D], f32, kind="ExternalInput")
    h_out = nc.dram_tensor("out", [S, V], bf16, kind="ExternalOutput")

    def bcast(handle, n, offset=0):
        ap = handle[:]
        return bass.AP(tensor=ap.tensor, offset=offset, ap=[[0, P], [1, n]])

    with TileContext(nc) as tc:
        import contextlib
        ctx = contextlib.ExitStack()
        with ctx:
            const = ctx.enter_context(tc.tile_pool(name="const", bufs=1))
            posp = ctx.enter_context(tc.tile_pool(name="posp", bufs=2))
            xb_p = ctx.enter_context(tc.tile_pool(name="xb", bufs=2))
            xt_p = ctx.enter_context(tc.tile_pool(name="xt", bufs=2))
            qk_p = ctx.enter_context(tc.tile_pool(name="qk", bufs=2))
            v_p = ctx.enter_context(tc.tile_pool(name="vp", bufs=2))
            k2t_p = ctx.enter_context(tc.tile_pool(name="k2t", bufs=1))
            pb_p = ctx.enter_context(tc.tile_pool(name="pb", bufs=4))
            pt_p = ctx.enter_context(tc.tile_pool(name="pt", bufs=1))
            xpre_p = ctx.enter_context(tc.tile_pool(name="xpre", bufs=2))
            stat_p = ctx.enter_context(tc.tile_pool(name="stat", bufs=4))
            wts_p = ctx.enter_context(tc.tile_pool(name="wts", bufs=3))
            wp_p = ctx.enter_context(tc.tile_pool(name="wpp", bufs=4))
            bp_p = ctx.enter_context(tc.tile_pool(name="bpp", bufs=2))
            osb_p = ctx.enter_context(tc.tile_pool(name="osb", bufs=6))
            ps = ctx.enter_context(tc.tile_pool(name="ps", bufs=8, space="PSUM"))

            ident = const.tile([P, P], bf16)
            make_identity(nc, ident)
            trimask = const.tile([P, P], f32)
            nc.gpsimd.memset(trimask, 0.0)
            nc.gpsimd.affine_select(
                out=trimask, in_=trimask, compare_op=ALU.is_ge, fill=-1e10,
                base=0, pattern=[[-1, P]], channel_multiplier=1)

            # ---- latency-critical DMAs first: tok, img, wk2, pos, gather ----
            tok_sb = const.tile([P, ST], i32)
            nc.sync.dma_start(out=tok_sb,
                              in_=h_tok[:].rearrange("(a p) -> p a", p=P))
            img_sb = const.tile([P, DIT, NI], bf16)
            nc.scalar.dma_start(out=img_sb, in_=h_img[:])
            wk2_sb = wts_p.tile([P, DIT, D], bf16, tag="wts", name="wk2")
            nc.scalar.dma_start(out=wk2_sb[:, :, :512], in_=h_wk2[:, :, :512])
            nc.scalar.dma_start(out=wk2_sb[:, :, 512:], in_=h_wk2[:, :, 512:])
            wv2_sb = wts_p.tile([P, DIT, D], bf16, tag="wts", name="wv2")
            nc.scalar.dma_start(out=wv2_sb[:, :, :512], in_=h_wv2[:, :, :512])
            nc.sync.dma_start(out=wv2_sb[:, :, 512:], in_=h_wv2[:, :, 512:])

            xrows = xb_p.tile([P, ST, D], bf16, tag="xb", name="xrows")
            for a in range(ST):
                nc.gpsimd.indirect_dma_start(
                    out=xrows[:, a, :], out_offset=None, in_=h_table[:],
                    in_offset=bass.IndirectOffsetOnAxis(ap=tok_sb[:, a:a + 1],
                                                        axis=0))
            # big self-attn weights: issued on gpsimd AFTER the gathers so the
            # early HBM window goes to gather/pos/img/wk2
            wq1_sb = wts_p.tile([P, DT, D], bf16, tag="wts", name="wq1")
            nc.gpsimd.dma_start(out=wq1_sb, in_=h_wq1[:])

            # ---- x0 = gather + pos ----
            epst = const.tile([P, 1], f32)
            nc.vector.memset(epst, EPS)
            x0b = xb_p.tile([P, ST, D], bf16, tag="xb")
            for a in range(ST):
                post = posp.tile([P, D], bf16, tag="pos")
                nc.sync.dma_start(out=post, in_=h_pos[a * P:(a + 1) * P, :])
                nc.vector.tensor_tensor(out=x0b[:, a, :], in0=xrows[:, a, :],
                                        in1=post, op=ALU.add)
            bq1s = const.tile([P, DT], f32)
            bk1s = const.tile([P, DT], f32)
            bq2s = const.tile([P, DT], f32)
            bk2s = const.tile([P, DT], f32)
            for t, h in ((bk2s, h_bk2), (bq1s, h_bq1), (bk1s, h_bk1),
                         (bq2s, h_bq2)):
                nc.sync.dma_start(out=t, in_=h[:])
            g1b = const.tile([P, D], f32)
            b1b = const.tile([P, D], f32)
            bv1b = const.tile([P, D], f32)
            bv2b = const.tile([P, D], f32)
            for t, h in ((bv2b, h_bv2), (bv1b, h_bv1), (g1b, h_g1),
                         (b1b, h_b1)):
                nc.sync.dma_start(out=t, in_=bcast(h, D))

            # ---- PE warmup: K2T / V2t (depend only on img + wk2/wv2) ----
            K2T = k2t_p.tile([P, DT, NI], bf16, tag="k2t")
            for m in range(DT):
                pm = ps.tile([P, 512], f32, tag="ps", name="k2ps")
                for k in range(DIT):
                    nc.tensor.matmul(pm[:, :NI],
                                     lhsT=wk2_sb[:, k, m * P:(m + 1) * P],
                                     rhs=img_sb[:, k, :],
                                     start=(k == 0), stop=(k == DIT - 1))
                nc.scalar.activation(out=K2T[:, m, :], in_=pm[:, :NI],
                                     func=ACT_F.Identity,
                                     bias=bk2s[:, m:m + 1], scale=1.0)
            # wk1 evicts wk2's slot: safe, K2T (wk2's readers) emitted above
            wk1_sb = wts_p.tile([P, DT, D], bf16, tag="wts", name="wk1")
            nc.gpsimd.dma_start(out=wk1_sb, in_=h_wk1[:])

            V2t = v_p.tile([P, NIT, D], bf16, tag="v")
            for a in range(NIT):
                pa = P if a == 0 else NI2
                for nh in range(2):
                    pm = ps.tile([P, 512], f32, tag="ps")
                    for k in range(DIT):
                        nc.tensor.matmul(
                            pm[:pa, :], lhsT=img_sb[:, k, a * P:a * P + pa],
                            rhs=wv2_sb[:, k, nh * 512:(nh + 1) * 512],
                            start=(k == 0), stop=(k == DIT - 1))
                    nc.vector.tensor_tensor(
                        out=V2t[:pa, a, nh * 512:(nh + 1) * 512], in0=pm[:pa, :],
                        in1=bv2b[:pa, nh * 512:(nh + 1) * 512], op=ALU.add)
            # wv1 evicts wv2's slot: safe, V2t (wv2's readers) emitted above
            wv1_sb = wts_p.tile([P, DT, D], bf16, tag="wts", name="wv1")
            nc.scalar.dma_start(out=wv1_sb, in_=h_wv1[:])

            def copy_sc(out, in_):
                nc.scalar.activation(out=out, in_=in_, func=ACT_F.Identity)

            def transpose_x(xb_tile, tag, on_scalar=False):
                """[P, ST, D] bf16 (seq-partition) -> [P, DT, S] bf16."""
                xt = xt_p.tile([P, DT, S], bf16, tag="xt", name=tag)
                for db in range(DT):
                    tp = ps.tile([P, 512], bf16, tag="ps", name="tp")
                    for a in range(ST):
                        nc.tensor.transpose(
                            out=tp[:, a * P:(a + 1) * P],
                            in_=xb_tile[:, a, db * P:(db + 1) * P],
                            identity=ident)
                        dst = xt[:, db, a * P:(a + 1) * P]
                        srcp = tp[:, a * P:(a + 1) * P]
                        if on_scalar:
                            copy_sc(dst, srcp)
                        else:
                            nc.vector.tensor_copy(out=dst, in_=srcp)
                return xt

            x0T = transpose_x(x0b, "x0t")

            # ---- projections ----
            def proj_T(w_sb, b_sb, rhsT, name):
                """QT/KT-style: out[P, DT, S] bf16 = (W.T @ x.T) + b."""
                o = qk_p.tile([P, DT, S], bf16, tag="qk", name=name)
                for m in range(DT):
                    pm = ps.tile([P, 512], f32, tag="ps", name="pm")
                    for k in range(DT):
                        nc.tensor.matmul(pm, lhsT=w_sb[:, k, m * P:(m + 1) * P],
                                         rhs=rhsT[:, k, :],
                                         start=(k == 0), stop=(k == DT - 1))
                    nc.scalar.activation(out=o[:, m, :], in_=pm,
                                         func=ACT_F.Identity,
                                         bias=b_sb[:, m:m + 1], scale=1.0)
                return o

            QT = proj_T(wq1_sb, bq1s, x0T, "qt")
            # wq2 evicts wq1's slot: safe, QT (wq1's readers) emitted above
            wq2_sb = wts_p.tile([P, DT, D], bf16, tag="wts", name="wq2")
            nc.scalar.dma_start(out=wq2_sb, in_=h_wq2[:])
            KT = proj_T(wk1_sb, bk1s, x0T, "kt")

            Vt = v_p.tile([P, ST, D], bf16, tag="v")
            for a in range(ST):
                for nh in range(2):
                    pm = ps.tile([P, 512], f32, tag="ps")
                    for k in range(DT):
                        nc.tensor.matmul(
                            pm, lhsT=x0T[:, k, a * P:(a + 1) * P],
                            rhs=wv1_sb[:, k, nh * 512:(nh + 1) * 512],
                            start=(k == 0), stop=(k == DT - 1))
                    nc.vector.tensor_tensor(
                        out=Vt[:, a, nh * 512:(nh + 1) * 512], in0=pm,
                        in1=bv1b[:, nh * 512:(nh + 1) * 512], op=ALU.add)

            # ---- causal self-attention: scores + softmax (all qt), then AV --
            Pbs = []
            rinv1 = stat_p.tile([P, ST], f32, tag="rinv")
            for qt in range(ST):
                width = (qt + 1) * P
                pm = ps.tile([P, 512], f32, tag="ps")
                for k in range(DT):
                    nc.tensor.matmul(pm[:, :width],
                                     lhsT=QT[:, k, qt * P:(qt + 1) * P],
                                     rhs=KT[:, k, :width],
                                     start=(k == 0), stop=(k == DT - 1))
                # in-place causal mask on the diagonal block (PSUM RMW)
                nc.vector.tensor_tensor(out=pm[:, qt * P:width],
                                        in0=pm[:, qt * P:width], in1=trimask,
                                        op=ALU.add)
                Pb = pb_p.tile([P, 512], bf16, tag="pb", name=f"pb{qt}")
                nc.scalar.activation(out=Pb[:, :width], in_=pm[:, :width],
                                     func=ACT_F.Exp, bias=0.0, scale=1.0,
                                     accum_out=rinv1[:, qt:qt + 1])
                Pbs.append(Pb)

            def layernorm(xpre, out_sl, gb, bb):
                """xpre [P, D] f32 -> out_sl [P, D] bf16; gb None = no affine."""
                stats = stat_p.tile([P, 2, 6], f32, tag="bnst")
                for sg in range(2):
                    nc.vector.bn_stats(out=stats[:, sg, :],
                                       in_=xpre[:, sg * 512:(sg + 1) * 512])
                mv = stat_p.tile([P, 2], f32, tag="bnmv")
                nc.vector.bn_aggr(out=mv, in_=stats)
                rstd = stat_p.tile([P, 1], f32, tag="rstd")
                nc.scalar.activation(out=rstd, in_=mv[:, 1:2], func=ACT_F.Sqrt,
                                     bias=epst, scale=1.0)
                nc.vector.reciprocal(out=rstd, in_=rstd)
                nmr = stat_p.tile([P, 1], f32, tag="nmr")
                nc.vector.tensor_tensor(out=nmr, in0=mv[:, 0:1], in1=rstd,
                                        op=ALU.mult)
                nc.scalar.mul(nmr, nmr, -1.0)
                if gb is None:
                    nc.scalar.activation(out=out_sl, in_=xpre,
                                         func=ACT_F.Identity,
                                         bias=nmr, scale=rstd)
                else:
                    nc.scalar.activation(out=xpre, in_=xpre,
                                         func=ACT_F.Identity,
                                         bias=nmr, scale=rstd)
                    nc.vector.tensor_tensor(out=xpre, in0=xpre, in1=gb,
                                            op=ALU.mult)
                    nc.vector.tensor_tensor(out=out_sl, in0=xpre, in1=bb,
                                            op=ALU.add)

            PT = pt_p.tile([P, ST, S], bf16, tag="pt")
            x1b = xb_p.tile([P, ST, D], bf16, tag="xb")
            for qt in range(ST):
                width = (qt + 1) * P
                x0s = xpre_p.tile([P, D], bf16, tag="xpre", name=f"x0s{qt}")
                nc.scalar.activation(out=x0s, in_=x0b[:, qt, :],
                                     func=ACT_F.Identity,
                                     scale=rinv1[:, qt:qt + 1])
                for kt in range(qt + 1):
                    tp = ps.tile([P, 512], bf16, tag="ps", name="tp")
                    nc.tensor.transpose(out=tp[:, :P],
                                        in_=Pbs[qt][:, kt * P:(kt + 1) * P],
                                        identity=ident)
                    copy_sc(PT[:, kt, qt * P:(qt + 1) * P], tp[:, :P])
                pmh = []
                for nh in range(2):
                    pm = ps.tile([P, 512], f32, tag="ps")
                    for kt in range(qt + 1):
                        nc.tensor.matmul(pm, lhsT=PT[:, kt, qt * P:(qt + 1) * P],
                                         rhs=Vt[:, kt, nh * 512:(nh + 1) * 512],
                                         start=(kt == 0), stop=False)
                    nc.tensor.matmul(pm, lhsT=ident,
                                     rhs=x0s[:, nh * 512:(nh + 1) * 512],
                                     start=False, stop=True)
                    pmh.append(pm)
                stats = stat_p.tile([P, 2, 6], f32, tag="bnst")
                for sg in range(2):
                    nc.vector.bn_stats(out=stats[:, sg, :], in_=pmh[sg])
                mv = stat_p.tile([P, 2], f32, tag="bnmv")
                nc.vector.bn_aggr(out=mv, in_=stats)
                rstd = stat_p.tile([P, 1], f32, tag="rstd")
                nc.scalar.activation(out=rstd, in_=mv[:, 1:2], func=ACT_F.Sqrt,
                                     bias=epst, scale=1.0)
                nc.vector.reciprocal(out=rstd, in_=rstd)
                nmr = stat_p.tile([P, 1], f32, tag="nmr")
                nc.vector.scalar_tensor_tensor(out=nmr, in0=mv[:, 0:1],
                                               scalar=-1.0, in1=rstd,
                                               op0=ALU.mult, op1=ALU.mult)
                xsc = xpre_p.tile([P, D], bf16, tag="xpre")
                for sg in range(2):
                    nc.scalar.activation(
                        out=xsc[:, sg * 512:(sg + 1) * 512],
                        in_=pmh[sg], func=ACT_F.Identity,
                        bias=nmr, scale=rstd)
                nc.gpsimd.tensor_tensor(out=xsc, in0=xsc, in1=g1b,
                                        op=ALU.mult)
                nc.gpsimd.tensor_tensor(out=x1b[:, qt, :], in0=xsc, in1=b1b,
                                        op=ALU.add)

            x1T = transpose_x(x1b, "x1t", on_scalar=True)

            # ---- cross attention: Q2, scores2 + softmax, then AV2 ----
            Q2T = proj_T(wq2_sb, bq2s, x1T, "q2t")

            P2bs = []
            rinv2 = stat_p.tile([P, ST], f32, tag="rinv")
            for qt in range(ST):
                pm = ps.tile([P, 512], f32, tag="ps")
                for k in range(DT):
                    nc.tensor.matmul(pm[:, :NI],
                                     lhsT=Q2T[:, k, qt * P:(qt + 1) * P],
                                     rhs=K2T[:, k, :],
                                     start=(k == 0), stop=(k == DT - 1))
                P2b = pb_p.tile([P, 512], bf16, tag="pb", name=f"p2b{qt}")
                nc.scalar.activation(out=P2b[:, :NI], in_=pm[:, :NI],
                                     func=ACT_F.Exp, bias=0.0, scale=1.0,
                                     accum_out=rinv2[:, qt:qt + 1])
                P2bs.append(P2b)

            PT2 = pt_p.tile([P, NIT, S], bf16, tag="pt")
            x2b = xb_p.tile([P, ST, D], bf16, tag="xb")
            for qt in range(ST):
                tp = ps.tile([P, 512], bf16, tag="ps", name="tp")
                nc.tensor.transpose(out=tp[:, :P],
                                    in_=P2bs[qt][:, :P], identity=ident)
                copy_sc(PT2[:, 0, qt * P:(qt + 1) * P], tp[:, :P])
                tp = ps.tile([P, 512], bf16, tag="ps", name="tp")
                nc.tensor.transpose(out=tp[:NI2, :P],
                                    in_=P2bs[qt][:, P:NI], identity=ident)
                copy_sc(PT2[:NI2, 1, qt * P:(qt + 1) * P], tp[:NI2, :P])
                xpre = xpre_p.tile([P, D], f32, tag="xpre")
                for nh in range(2):
                    pm = ps.tile([P, 512], f32, tag="ps")
                    nc.tensor.matmul(pm, lhsT=PT2[:, 0, qt * P:(qt + 1) * P],
                                     rhs=V2t[:, 0, nh * 512:(nh + 1) * 512],
                                     start=True, stop=False)
                    nc.tensor.matmul(pm,
                                     lhsT=PT2[:NI2, 1, qt * P:(qt + 1) * P],
                                     rhs=V2t[:NI2, 1, nh * 512:(nh + 1) * 512],
                                     start=False, stop=True)
                    nc.vector.scalar_tensor_tensor(
                        out=xpre[:, nh * 512:(nh + 1) * 512],
                        in0=x1b[:, qt, nh * 512:(nh + 1) * 512],
                        scalar=rinv2[:, qt:qt + 1], in1=pm,
                        op0=ALU.mult, op1=ALU.add)
                layernorm(xpre, x2b[:, qt, :], None, None)

            x2T = transpose_x(x2b, "x2t", on_scalar=True)

            # ---- vocab projection, streamed in CN-column chunks ----
            def vocab_group(chunks, g_off, width_last):
                """chunks: list of (c_idx, width). One osb strip per qt."""
                total_w = sum(w for _, w in chunks)
                bp_bc = bp_p.tile([P, GRP * CN], bf16, tag="bp")
                nc.scalar.dma_start(out=bp_bc[:, :total_w],
                                    in_=bcast(h_bp, total_w, offset=g_off))
                osb = [osb_p.tile([P, GRP * CN], bf16, tag="osb",
                                  name=f"osb_{g_off}_{q}")
                       for q in range(ST)]
                col = 0
                for ci, (c, w) in enumerate(chunks):
                    wp_sb = wp_p.tile([P, DT, CN], bf16, tag="wp")
                    src = h_wp[c] if c < NFULL else h_wpl[:]
                    dma_eng = nc.sync if c % 2 == 0 else nc.scalar
                    dma_eng.dma_start(out=wp_sb[:, :, :w], in_=src)
                    for qt in range(ST):
                        pm = ps.tile([P, 512], f32, tag="ps")
                        for k in range(DT):
                            nc.tensor.matmul(
                                pm[:, :w], lhsT=x2T[:, k, qt * P:(qt + 1) * P],
                                rhs=wp_sb[:, k, :w],
                                start=(k == 0), stop=(k == DT - 1))
                        nc.vector.tensor_tensor(
                            out=osb[qt][:, col:col + w], in0=pm[:, :w],
                            in1=bp_bc[:, col:col + w], op=ALU.add)
                    col += w
                col = 0
                for ci, (c, w) in enumerate(chunks):
                    for qt in range(ST):
                        out_eng = nc.sync if qt < 2 else nc.scalar
                        out_eng.dma_start(
                            out=h_out[qt * P:(qt + 1) * P,
                                      g_off + col:g_off + col + w],
                            in_=osb[qt][:, col:col + w])
                    col += w

            vocab_group([(NFULL, CLAST)], NFULL * CN, CLAST)
            for g in range(NGRP):
                vocab_group([(g * GRP + cc, CN) for cc in range(GRP)],
                            g * GRP * CN, CN)

    nc.compile()
    return nc


def _tile_sq(w, kt):
    """[K, N] -> [128, K//128, N] contiguous."""
    k, n = w.shape
    assert k == kt * P
    return np.ascontiguousarray(
        w.reshape(kt, P, n).transpose(1, 0, 2)).astype(BF16)


def _prep_inputs(inputs):
    g = lambda name: np.asarray(inputs[name], dtype=np.float32)
    tokens = np.asarray(inputs["tokens"]).astype(np.int32)
    img = g("img_emb")

    # positional encoding (same closed form as the model definition)
    posn = np.arange(S)[:, None].astype(np.float32)
    i = np.arange(0, D, 2).astype(np.float32)
    ang = posn / np.power(10000.0, i / D)
    pos = np.zeros((S, D), dtype=np.float32)
    pos[:, 0::2] = np.sin(ang)
    pos[:, 1::2] = np.cos(ang)

    # fold LN2 affine into the vocab projection: out = n@(g2*Wp) + (b2@Wp+bp)
    wp = g("Wp") * g("g2")[:, None]          # [D, V]
    bp_eff = (g("b2") @ g("Wp") + g("bp")).astype(BF16)
    wp_t = np.ascontiguousarray(
        wp.reshape(DT, P, V).transpose(1, 0, 2)).astype(BF16)  # [P, DT, V]
    wp_main = np.ascontiguousarray(
        wp_t[:, :, :NFULL * CN].reshape(P, DT, NFULL, CN)
        .transpose(2, 0, 1, 3))              # [NFULL, P, DT, CN]
    wp_last = np.ascontiguousarray(wp_t[:, :, NFULL * CN:])  # [P, DT, CLAST]

    def bias_tiled(b):
        return np.ascontiguousarray(b.reshape(DT, P).T).astype(np.float32)

    shared = {
        "table": g("emb_table").astype(BF16),
        "pos": pos.astype(BF16),
        "wq1": _tile_sq(g("Wq1") * SCALE, DT),
        "wk1": _tile_sq(g("Wk1"), DT),
        "wv1": _tile_sq(g("Wv1"), DT),
        "wq2": _tile_sq(g("Wq2") * SCALE, DT),
        "wk2": _tile_sq(g("Wk2"), DIT),
        "wv2": _tile_sq(g("Wv2"), DIT),
        "wp": wp_main,
        "wpl": wp_last,
        "bq1": bias_tiled(g("bq1") * SCALE),
        "bk1": bias_tiled(g("bk1")),
        "bq2": bias_tiled(g("bq2") * SCALE),
        "bk2": bias_tiled(g("bk2")),
        "bv1": g("bv1"),
        "bv2": g("bv2"),
        "bp": bp_eff,
        "g1": g("g1"), "b1": g("b1"),
    }
    in_maps = []
    for c in range(N_CORES):
        m = dict(shared)
        m["tok"] = np.ascontiguousarray(tokens[c])
        m["img_t"] = np.ascontiguousarray(
            img[c].T.reshape(DIT, P, NI).transpose(1, 0, 2)).astype(BF16)
        in_maps.append(m)
    return in_maps


def _ensure_axon_hooks():
    """bass_utils imports antenv.axon_hooks when BASS_TRACE is set; stub it
    if the module is absent so tracing degrades instead of crashing."""
    try:
        import antenv.axon_hooks  # noqa: F401
    except ImportError:
        import types
        mod = types.ModuleType("antenv.axon_hooks")
        mod.get_axon_ntff_profile_hook = lambda: None
        mod.set_axon_ntff_profile_hook = lambda h: None
        sys.modules["antenv.axon_hooks"] = mod


def kernel(**inputs):
    global LAST_RESULTS
    _ensure_axon_hooks()
    from concourse.bass_utils import run_bass_kernel_spmd

    if "nc" not in _CACHE:
        _CACHE["nc"] = _build_program()
    nc = _CACHE["nc"]

    in_maps = _prep_inputs(inputs)
    res = run_bass_kernel_spmd(nc, in_maps, core_ids=list(range(N_CORES)))
    LAST_RESULTS = res
    out = np.stack([res.results[c]["out"].astype(np.float32)
                    for c in range(N_CORES)])
    return out


# revision 41
# speedup vs baseline: 1.0110x; 1.0110x over previous
"""Trainium2 Bass kernel for an 8-batch image-conditioned decoder layer.

Strategy: pure data-parallel over the batch — core c computes batch element c
end-to-end (embedding gather, causal self-attention, cross-attention over the
image tokens, both layernorms, vocab projection). No collectives.

v6 = v1 baseline schedule plus: exact 32000-wide vocab projection (62 full
chunks + one 256 chunk, no padding), LN2 affine folded into Wp/bp on the host,
in-place PSUM causal masking, and K2/V2 warmup first with weight DMAs
staggered (pool-slot sequencing) so the early HBM window serves the
latency-critical gather/pos/img loads.
"""

import os
import sys

for _p in ("/opt/trn_rl_repo", "/root/.axon_site/_ro/trn_rl_repo"):
    if os.path.isdir(_p) and _p not in sys.path:
        sys.path.append(_p)

import numpy as np
import ml_dtypes

BF16 = ml_dtypes.bfloat16

# Problem dims (hardcoded per spec)
V, D, DI, S, B, NI = 32000, 1024, 768, 512, 8, 197
EPS = 1e-5
P = 128
ST = S // P          # 4 seq tiles
DT = D // P          # 8 model-dim tiles
DIT = DI // P        # 6 image-dim tiles
NIT = 2              # image tokens: 197 -> 2 partition tiles (128 + 69)
NI2 = NI - P         # 69
CN = 512             # vocab chunk width
NFULL = V // CN      # 62 full chunks
CLAST = V - NFULL * CN   # 256
GRP = 2              # chunks per output strip
NGRP = NFULL // GRP  # 31
N_CORES = 8
SCALE = 1.0 / float(np.sqrt(np.float32(D)))

_CACHE = {}
LAST_RESULTS = None


def _build_program():
    import concourse.bacc as bacc
    import concourse.bass as bass
    import concourse.mybir as mybir
    from concourse.masks import make_identity
    from concourse.tile import TileContext

    f32 = mybir.dt.float32
    bf16 = mybir.dt.bfloat16
    i32 = mybir.dt.int32
    X = mybir.AxisListType.X
    ALU = mybir.AluOpType
    ACT_F = mybir.ActivationFunctionType

    nc = bacc.Bacc("TRN2", target_bir_lowering=False, debug=False,
                   num_devices=N_CORES)

    # ---- I/O ----
    h_tok = nc.dram_tensor("tok", [S], i32, kind="ExternalInput")
    h_table = nc.dram_tensor("table", [V, D], bf16, kind="ExternalInput")
    h_pos = nc.dram_tensor("pos", [S, D], bf16, kind="ExternalInput")
    h_img = nc.dram_tensor("img_t", [P, DIT, NI], bf16, kind="ExternalInput")
    h_wq1 = nc.dram_tensor("wq1", [P, DT, D], bf16, kind="ExternalInput")
    h_wk1 = nc.dram_tensor("wk1", [P, DT, D], bf16, kind="ExternalInput")
    h_wv1 = nc.dram_tensor("wv1", [P, DT, D], bf16, kind="ExternalInput")
    h_wq2 = nc.dram_tensor("wq2", [P, DT, D], bf16, kind="ExternalInput")
    h_wk2 = nc.dram_tensor("wk2", [P, DIT, D], bf16, kind="ExternalInput")
    h_wv2 = nc.dram_tensor("wv2", [P, DIT, D], bf16, kind="ExternalInput")
    h_wp = nc.dram_tensor("wp", [NFULL, P, DT, CN], bf16, kind="ExternalInput")
    h_wpl = nc.dram_tensor("wpl", [P, DT, CLAST], bf16, kind="ExternalInput")
    h_bq1 = nc.dram_tensor("bq1", [P, DT], f32, kind="ExternalInput")
    h_bk1 = nc.dram_tensor("bk1", [P, DT], f32, kind="ExternalInput")
    h_bq2 = nc.dram_tensor("bq2", [P, DT], f32, kind="ExternalInput")
    h_bk2 = nc.dram_tensor("bk2", [P, DT], f32, kind="ExternalInput")
    h_bv1 = nc.dram_tensor("bv1", [D], f32, kind="ExternalInput")
    h_bv2 = nc.dram_tensor("bv2", [D], f32, kind="ExternalInput")
    h_bp = nc.dram_tensor("bp", [V], bf16, kind="ExternalInput")
    h_g1 = nc.dram_tensor("g1", [D], f32, kind="ExternalInput")
    h_b1 = nc.dram_tensor("b1", [D], f32, kind="ExternalInput")
    h_out = nc.dram_tensor("out", [S, V], bf16, kind="ExternalOutput")

    def bcast(handle, n, offset=0):
        ap = handle[:]
        return bass.AP(tensor=ap.tensor, offset=offset, ap=[[0, P], [1, n]])

    with TileContext(nc) as tc:
        import contextlib
        ctx = contextlib.ExitStack()
        with ctx:
            const = ctx.enter_context(tc.tile_pool(name="const", bufs=1))
            posp = ctx.enter_context(tc.tile_pool(name="posp", bufs=2))
            xb_p = ctx.enter_context(tc.tile_pool(name="xb", bufs=2))
            xt_p = ctx.enter_context(tc.tile_pool(name="xt", bufs=2))
            qk_p = ctx.enter_context(tc.tile_pool(name="qk", bufs=2))
            v_p = ctx.enter_context(tc.tile_pool(name="vp", bufs=2))
            k2t_p = ctx.enter_context(tc.tile_pool(name="k2t", bufs=1))
            pb_p = ctx.enter_context(tc.tile_pool(name="pb", bufs=4))
            pt_p = ctx.enter_context(tc.tile_pool(name="pt", bufs=1))
            xpre_p = ctx.enter_context(tc.tile_pool(name="xpre", bufs=2))
            stat_p = ctx.enter_context(tc.tile_pool(name="stat", bufs=4))
            wts_p = ctx.enter_context(tc.tile_pool(name="wts", bufs=3))
            wp_p = ctx.enter_context(tc.tile_pool(name="wpp", bufs=4))
            bp_p = ctx.enter_context(tc.tile_pool(name="bpp", bufs=2))
            osb_p = ctx.enter_context(tc.tile_pool(name="osb", bufs=6))
            ps = ctx.enter_context(tc.tile_pool(name="ps", bufs=8, space="PSUM"))

            ident = const.tile([P, P], bf16)
            make_identity(nc, ident)
            trimask = const.tile([P, P], f32)
            nc.gpsimd.memset(trimask, 0.0)
            nc.gpsimd.affine_select(
                out=trimask, in_=trimask, compare_op=ALU.is_ge, fill=-1e10,
                base=0, pattern=[[-1, P]], channel_multiplier=1)

            # ---- latency-critical DMAs first: tok, img, wk2, pos, gather ----
            tok_sb = const.tile([P, ST], i32)
            nc.sync.dma_start(out=tok_sb,
                              in_=h_tok[:].rearrange("(a p) -> p a", p=P))
            img_sb = const.tile([P, DIT, NI], bf16)
            nc.scalar.dma_start(out=img_sb, in_=h_img[:])
            wk2_sb = wts_p.tile([P, DIT, D], bf16, tag="wts", name="wk2")
            nc.scalar.dma_start(out=wk2_sb[:, :, :512], in_=h_wk2[:, :, :512])
            nc.scalar.dma_start(out=wk2_sb[:, :, 512:], in_=h_wk2[:, :, 512:])
            wv2_sb = wts_p.tile([P, DIT, D], bf16, tag="wts", name="wv2")
            nc.scalar.dma_start(out=wv2_sb[:, :, :512], in_=h_wv2[:, :, :512])
            nc.sync.dma_start(out=wv2_sb[:, :, 512:], in_=h_wv2[:, :, 512:])

            xrows = xb_p.tile([P, ST, D], bf16, tag="xb", name="xrows")
            for a in range(ST):
                nc.gpsimd.indirect_dma_start(
                    out=xrows[:, a, :], out_offset=None, in_=h_table[:],
                    in_offset=bass.IndirectOffsetOnAxis(ap=tok_sb[:, a:a + 1],
                                                        axis=0))
            # big self-attn weights: issued on gpsimd AFTER the gathers so the
            # early HBM window goes to gather/pos/img/wk2
            wq1_sb = wts_p.tile([P, DT, D], bf16, tag="wts", name="wq1")
            nc.gpsimd.dma_start(out=wq1_sb, in_=h_wq1[:])

            # ---- x0 = gather + pos ----
            epst = const.tile([P, 1], f32)
            nc.vector.memset(epst, EPS)
            x0b = xb_p.tile([P, ST, D], bf16, tag="xb")
            for a in range(ST):
                post = posp.tile([P, D], bf16, tag="pos")
                nc.sync.dma_start(out=post, in_=h_pos[a * P:(a + 1) * P, :])
                nc.vector.tensor_tensor(out=x0b[:, a, :], in0=xrows[:, a, :],
                                        in1=post, op=ALU.add)
            bq1s = const.tile([P, DT], f32)
            bk1s = const.tile([P, DT], f32)
            bq2s = const.tile([P, DT], f32)
            bk2s = const.tile([P, DT], f32)
            for t, h in ((bk2s, h_bk2), (bq1s, h_bq1), (bk1s, h_bk1),
                         (bq2s, h_bq2)):
                nc.sync.dma_start(out=t, in_=h[:])
            g1b = const.tile([P, D], f32)
            b1b = const.tile([P, D], f32)
            bv1b = const.tile([P, D], f32)
            bv2b = const.tile([P, D], f32)
            for t, h in ((bv2b, h_bv2), (bv1b, h_bv1), (g1b, h_g1),
                         (b1b, h_b1)):
                nc.sync.dma_start(out=t, in_=bcast(h, D))

            # ---- PE warmup: K2T / V2t (depend only on img + wk2/wv2) ----
            K2T = k2t_p.tile([P, DT, NI], bf16, tag="k2t")
            for m in range(DT):
                pm = ps.tile([P, 512], f32, tag="ps", name="k2ps")
                for k in range(DIT):
                    nc.tensor.matmul(pm[:, :NI],
                                     lhsT=wk2_sb[:, k, m * P:(m + 1) * P],
                                     rhs=img_sb[:, k, :],
                                     start=(k == 0), stop=(k == DIT - 1))
                nc.scalar.activation(out=K2T[:, m, :], in_=pm[:, :NI],
                                     func=ACT_F.Identity,
                                     bias=bk2s[:, m:m + 1], scale=1.0)
            # wk1 evicts wk2's slot: safe, K2T (wk2's readers) emitted above
            wk1_sb = wts_p.tile([P, DT, D], bf16, tag="wts", name="wk1")
            nc.gpsimd.dma_start(out=wk1_sb, in_=h_wk1[:])

            V2t = v_p.tile([P, NIT, D], bf16, tag="v")
            for a in range(NIT):
                pa = P if a == 0 else NI2
                for nh in range(2):
                    pm = ps.tile([P, 512], f32, tag="ps")
                    for k in range(DIT):
                        nc.tensor.matmul(
                            pm[:pa, :], lhsT=img_sb[:, k, a * P:a * P + pa],
                            rhs=wv2_sb[:, k, nh * 512:(nh + 1) * 512],
                            start=(k == 0), stop=(k == DIT - 1))
                    nc.vector.tensor_tensor(
                        out=V2t[:pa, a, nh * 512:(nh + 1) * 512], in0=pm[:pa, :],
                        in1=bv2b[:pa, nh * 512:(nh + 1) * 512], op=ALU.add)
            # wv1 evicts wv2's slot: safe, V2t (wv2's readers) emitted above
            wv1_sb = wts_p.tile([P, DT, D], bf16, tag="wts", name="wv1")
            nc.scalar.dma_start(out=wv1_sb, in_=h_wv1[:])

            def copy_sc(out, in_):
                nc.scalar.activation(out=out, in_=in_, func=ACT_F.Identity)

            def transpose_x(xb_tile, tag, on_scalar=False):
                """[P, ST, D] bf16 (seq-partition) -> [P, DT, S] bf16."""
                xt = xt_p.tile([P, DT, S], bf16, tag="xt", name=tag)
                for db in range(DT):
                    tp = ps.tile([P, 512], bf16, tag="ps", name="tp")
                    for a in range(ST):
                        nc.tensor.transpose(
                            out=tp[:, a * P:(a + 1) * P],
                            in_=xb_tile[:, a, db * P:(db + 1) * P],
                            identity=ident)
                        dst = xt[:, db, a * P:(a + 1) * P]
                        srcp = tp[:, a * P:(a + 1) * P]
                        if on_scalar:
                            copy_sc(dst, srcp)
                        else:
                            nc.vector.tensor_copy(out=dst, in_=srcp)
                return xt

            x0T = transpose_x(x0b, "x0t")

            # ---- projections ----
            def proj_T(w_sb, b_sb, rhsT, name):
                """QT/KT-style: out[P, DT, S] bf16 = (W.T @ x.T) + b."""
                o = qk_p.tile([P, DT, S], bf16, tag="qk", name=name)
                for m in range(DT):
                    pm = ps.tile([P, 512], f32, tag="ps", name="pm")
                    for k in range(DT):
                        nc.tensor.matmul(pm, lhsT=w_sb[:, k, m * P:(m + 1) * P],
                                         rhs=rhsT[:, k, :],
                                         start=(k == 0), stop=(k == DT - 1))
                    nc.scalar.activation(out=o[:, m, :], in_=pm,
                                         func=ACT_F.Identity,
                                         bias=b_sb[:, m:m + 1], scale=1.0)
                return o

            QT = proj_T(wq1_sb, bq1s, x0T, "qt")
            # wq2 evicts wq1's slot: safe, QT (wq1's readers) emitted above
            wq2_sb = wts_p.tile([P, DT, D], bf16, tag="wts", name="wq2")
            nc.scalar.dma_start(out=wq2_sb, in_=h_wq2[:])
            KT = proj_T(wk1_sb, bk1s, x0T, "kt")

            Vt = v_p.tile([P, ST, D], bf16, tag="v")
            for a in range(ST):
                for nh in range(2):
                    pm = ps.tile([P, 512], f32, tag="ps")
                    for k in range(DT):
                        nc.tensor.matmul(
                            pm, lhsT=x0T[:, k, a * P:(a + 1) * P],
                            rhs=wv1_sb[:, k, nh * 512:(nh + 1) * 512],
                            start=(k == 0), stop=(k == DT - 1))
                    nc.vector.tensor_tensor(
                        out=Vt[:, a, nh * 512:(nh + 1) * 512], in0=pm,
                        in1=bv1b[:, nh * 512:(nh + 1) * 512], op=ALU.add)

            # ---- causal self-attention: scores + softmax (all qt), then AV --
            Pbs = []
            rinv1 = stat_p.tile([P, ST], f32, tag="rinv")
            for qt in range(ST):
                width = (qt + 1) * P
                pm = ps.tile([P, 512], f32, tag="ps")
                for k in range(DT):
                    nc.tensor.matmul(pm[:, :width],
                                     lhsT=QT[:, k, qt * P:(qt + 1) * P],
                                     rhs=KT[:, k, :width],
                                     start=(k == 0), stop=(k == DT - 1))
                # in-place causal mask on the diagonal block (PSUM RMW)
                nc.vector.tensor_tensor(out=pm[:, qt * P:width],
                                        in0=pm[:, qt * P:width], in1=trimask,
                                        op=ALU.add)
                Pb = pb_p.tile([P, 512], bf16, tag="pb", name=f"pb{qt}")
                rsum = stat_p.tile([P, 1], f32, tag="rsum")
                nc.scalar.activation(out=Pb[:, :width], in_=pm[:, :width],
                                     func=ACT_F.Exp, bias=0.0, scale=1.0,
                                     accum_out=rsum)
                nc.vector.reciprocal(out=rinv1[:, qt:qt + 1], in_=rsum)
                Pbs.append(Pb)

            def layernorm(xpre, out_sl, gb, bb):
                """xpre [P, D] f32 -> out_sl [P, D] bf16; gb None = no affine."""
                stats = stat_p.tile([P, 2, 6], f32, tag="bnst")
                for sg in range(2):
                    nc.vector.bn_stats(out=stats[:, sg, :],
                                       in_=xpre[:, sg * 512:(sg + 1) * 512])
                mv = stat_p.tile([P, 2], f32, tag="bnmv")
                nc.vector.bn_aggr(out=mv, in_=stats)
                rstd = stat_p.tile([P, 1], f32, tag="rstd")
                nc.scalar.activation(out=rstd, in_=mv[:, 1:2], func=ACT_F.Sqrt,
                                     bias=epst, scale=1.0)
                nc.vector.reciprocal(out=rstd, in_=rstd)
                nmr = stat_p.tile([P, 1], f32, tag="nmr")
                nc.vector.tensor_tensor(out=nmr, in0=mv[:, 0:1], in1=rstd,
                                        op=ALU.mult)
                nc.scalar.mul(nmr, nmr, -1.0)
                if gb is None:
                    nc.scalar.activation(out=out_sl, in_=xpre,
                                         func=ACT_F.Identity,
                                         bias=nmr, scale=rstd)
                else:
                    nc.scalar.activation(out=xpre, in_=xpre,
                                         func=ACT_F.Identity,
                                         bias=nmr, scale=rstd)
                    nc.vector.tensor_tensor(out=xpre, in0=xpre, in1=gb,
                                            op=ALU.mult)
                    nc.vector.tensor_tensor(out=out_sl, in0=xpre, in1=bb,
                                            op=ALU.add)

            PT = pt_p.tile([P, ST, S], bf16, tag="pt")
            x1b = xb_p.tile([P, ST, D], bf16, tag="xb")
            for qt in range(ST):
                width = (qt + 1) * P
                Pbn = pb_p.tile([P, 512], bf16, tag="pbn", name=f"pbn{qt}")
                nc.scalar.activation(out=Pbn[:, :width], in_=Pbs[qt][:, :width],
                                     func=ACT_F.Identity,
                                     scale=rinv1[:, qt:qt + 1])
                for kt in range(qt + 1):
                    tp = ps.tile([P, 512], bf16, tag="ps", name="tp")
                    nc.tensor.transpose(out=tp[:, :P],
                                        in_=Pbn[:, kt * P:(kt + 1) * P],
                                        identity=ident)
                    copy_sc(PT[:, kt, qt * P:(qt + 1) * P], tp[:, :P])
                pmh = []
                for nh in range(2):
                    pm = ps.tile([P, 512], f32, tag="ps")
                    for kt in range(qt + 1):
                        nc.tensor.matmul(pm, lhsT=PT[:, kt, qt * P:(qt + 1) * P],
                                         rhs=Vt[:, kt, nh * 512:(nh + 1) * 512],
                                         start=(kt == 0), stop=False)
                    nc.tensor.matmul(pm, lhsT=ident,
                                     rhs=x0b[:, qt, nh * 512:(nh + 1) * 512],
                                     start=False, stop=True)
                    pmh.append(pm)
                stats = stat_p.tile([P, 2, 6], f32, tag="bnst")
                for sg in range(2):
                    nc.vector.bn_stats(out=stats[:, sg, :], in_=pmh[sg])
                mv = stat_p.tile([P, 2], f32, tag="bnmv")
                nc.vector.bn_aggr(out=mv, in_=stats)
                rstd = stat_p.tile([P, 1], f32, tag="rstd")
                nc.scalar.activation(out=rstd, in_=mv[:, 1:2], func=ACT_F.Sqrt,
                                     bias=epst, scale=1.0)
                nc.vector.reciprocal(out=rstd, in_=rstd)
                nmr = stat_p.tile([P, 1], f32, tag="nmr")
                nc.vector.scalar_tensor_tensor(out=nmr, in0=mv[:, 0:1],
                                               scalar=-1.0, in1=rstd,
                                               op0=ALU.mult, op1=ALU.mult)
                xsc = xpre_p.tile([P, D], bf16, tag="xpre")
                for sg in range(2):
                    nc.scalar.activation(
                        out=xsc[:, sg * 512:(sg + 1) * 512],
                        in_=pmh[sg], func=ACT_F.Identity,
                        bias=nmr, scale=rstd)
                nc.gpsimd.tensor_tensor(out=xsc, in0=xsc, in1=g1b,
                                        op=ALU.mult)
                nc.gpsimd.tensor_tensor(out=x1b[:, qt, :], in0=xsc, in1=b1b,
                                        op=ALU.add)

            x1T = transpose_x(x1b, "x1t", on_scalar=True)

            # ---- cross attention: Q2, scores2 + softmax, then AV2 ----
            Q2T = proj_T(wq2_sb, bq2s, x1T, "q2t")

            P2bs = []
            rinv2 = stat_p.tile([P, ST], f32, tag="rinv")
            for qt in range(ST):
                pm = ps.tile([P, 512], f32, tag="ps")
                for k in range(DT):
                    nc.tensor.matmul(pm[:, :NI],
                                     lhsT=Q2T[:, k, qt * P:(qt + 1) * P],
                                     rhs=K2T[:, k, :],
                                     start=(k == 0), stop=(k == DT - 1))
                P2b = pb_p.tile([P, 512], bf16, tag="pb", name=f"p2b{qt}")
                rsum = stat_p.tile([P, 1], f32, tag="rsum")
                nc.scalar.activation(out=P2b[:, :NI], in_=pm[:, :NI],
                                     func=ACT_F.Exp, bias=0.0, scale=1.0,
                                     accum_out=rsum)
                nc.vector.reciprocal(out=rinv2[:, qt:qt + 1], in_=rsum)
                P2bs.append(P2b)

            PT2 = pt_p.tile([P, NIT, S], bf16, tag="pt")
            x2b = xb_p.tile([P, ST, D], bf16, tag="xb")
            for qt in range(ST):
                tp = ps.tile([P, 512], bf16, tag="ps", name="tp")
                nc.tensor.transpose(out=tp[:, :P],
                                    in_=P2bs[qt][:, :P], identity=ident)
                copy_sc(PT2[:, 0, qt * P:(qt + 1) * P], tp[:, :P])
                tp = ps.tile([P, 512], bf16, tag="ps", name="tp")
                nc.tensor.transpose(out=tp[:NI2, :P],
                                    in_=P2bs[qt][:, P:NI], identity=ident)
                copy_sc(PT2[:NI2, 1, qt * P:(qt + 1) * P], tp[:NI2, :P])
                xpre = xpre_p.tile([P, D], f32, tag="xpre")
                for nh in range(2):
                    pm = ps.tile([P, 512], f32, tag="ps")
                    nc.tensor.matmul(pm, lhsT=PT2[:, 0, qt * P:(qt + 1) * P],
                                     rhs=V2t[:, 0, nh * 512:(nh + 1) * 512],
                                     start=True, stop=False)
                    nc.tensor.matmul(pm,
                                     lhsT=PT2[:NI2, 1, qt * P:(qt + 1) * P],
                                     rhs=V2t[:NI2, 1, nh * 512:(nh + 1) * 512],
                                     start=False, stop=True)
                    nc.vector.scalar_tensor_tensor(
                        out=xpre[:, nh * 512:(nh + 1) * 512], in0=pm,
                        scalar=rinv2[:, qt:qt + 1],
                        in1=x1b[:, qt, nh * 512:(nh + 1) * 512],
                        op0=ALU.mult, op1=ALU.add)
                layernorm(xpre, x2b[:, qt, :], None, None)

            x2T = transpose_x(x2b, "x2t", on_scalar=True)

            # ---- vocab projection, streamed in CN-column chunks ----
            def vocab_group(chunks, g_off, width_last):
                """chunks: list of (c_idx, width). One osb strip per qt."""
                total_w = sum(w for _, w in chunks)
                bp_bc = bp_p.tile([P, GRP * CN], bf16, tag="bp")
                nc.scalar.dma_start(out=bp_bc[:, :total_w],
                                    in_=bcast(h_bp, total_w, offset=g_off))
                osb = [osb_p.tile([P, GRP * CN], bf16, tag="osb",
                                  name=f"osb_{g_off}_{q}")
                       for q in range(ST)]
                col = 0
                for ci, (c, w) in enumerate(chunks):
                    wp_sb = wp_p.tile([P, DT, CN], bf16, tag="wp")
                    src = h_wp[c] if c < NFULL else h_wpl[:]
                    dma_eng = nc.sync if c % 2 == 0 else nc.scalar
                    dma_eng.dma_start(out=wp_sb[:, :, :w], in_=src)
                    for qt in range(ST):
                        pm = ps.tile([P, 512], f32, tag="ps")
                        for k in range(DT):
                            nc.tensor.matmul(
                                pm[:, :w], lhsT=x2T[:, k, qt * P:(qt + 1) * P],
                                rhs=wp_sb[:, k, :w],
                                start=(k == 0), stop=(k == DT - 1))
                        nc.vector.tensor_tensor(
                            out=osb[qt][:, col:col + w], in0=pm[:, :w],
                            in1=bp_bc[:, col:col + w], op=ALU.add)
                    col += w
                col = 0
                for ci, (c, w) in enumerate(chunks):
                    for qt in range(ST):
                        out_eng = nc.sync if qt < 2 else nc.scalar
                        out_eng.dma_start(
                            out=h_out[qt * P:(qt + 1) * P,
                                      g_off + col:g_off + col + w],
                            in_=osb[qt][:, col:col + w])
                    col += w

            vocab_group([(NFULL, CLAST)], NFULL * CN, CLAST)
            for g in range(NGRP):
                vocab_group([(g * GRP + cc, CN) for cc in range(GRP)],
                            g * GRP * CN, CN)

    nc.compile()
    return nc


def _tile_sq(w, kt):
    """[K, N] -> [128, K//128, N] contiguous."""
    k, n = w.shape
    assert k == kt * P
    return np.ascontiguousarray(
        w.reshape(kt, P, n).transpose(1, 0, 2)).astype(BF16)


def _prep_inputs(inputs):
    g = lambda name: np.asarray(inputs[name], dtype=np.float32)
    tokens = np.asarray(inputs["tokens"]).astype(np.int32)
    img = g("img_emb")

    # positional encoding (same closed form as the model definition)
    posn = np.arange(S)[:, None].astype(np.float32)
    i = np.arange(0, D, 2).astype(np.float32)
    ang = posn / np.power(10000.0, i / D)
    pos = np.zeros((S, D), dtype=np.float32)
    pos[:, 0::2] = np.sin(ang)
    pos[:, 1::2] = np.cos(ang)

    # fold LN2 affine into the vocab projection: out = n@(g2*Wp) + (b2@Wp+bp)
    wp = g("Wp") * g("g2")[:, None]          # [D, V]
    bp_eff = (g("b2") @ g("Wp") + g("bp")).astype(BF16)
    wp_t = np.ascontiguousarray(
        wp.reshape(DT, P, V).transpose(1, 0, 2)).astype(BF16)  # [P, DT, V]
    wp_main = np.ascontiguousarray(
        wp_t[:, :, :NFULL * CN].reshape(P, DT, NFULL, CN)
        .transpose(2, 0, 1, 3))              # [NFULL, P, DT, CN]
    wp_last = np.ascontiguousarray(wp_t[:, :, NFULL * CN:])  # [P, DT, CLAST]

    def bias_tiled(b):
        return np.ascontiguousarray(b.reshape(DT, P).T).astype(np.float32)

    shared = {
        "table": g("emb_table").astype(BF16),
        "pos": pos.astype(BF16),
        "wq1": _tile_sq(g("Wq1") * SCALE, DT),
        "wk1": _tile_sq(g("Wk1"), DT),
        "wv1": _tile_sq(g("Wv1"), DT),
        "wq2": _tile_sq(g("Wq2") * SCALE, DT),
        "wk2": _tile_sq(g("Wk2"), DIT),
        "wv2": _tile_sq(g("Wv2"), DIT),
        "wp": wp_main,
        "wpl": wp_last,
        "bq1": bias_tiled(g("bq1") * SCALE),
        "bk1": bias_tiled(g("bk1")),
        "bq2": bias_tiled(g("bq2") * SCALE),
        "bk2": bias_tiled(g("bk2")),
        "bv1": g("bv1"),
        "bv2": g("bv2"),
        "bp": bp_eff,
        "g1": g("g1"), "b1": g("b1"),
    }
    in_maps = []
    for c in range(N_CORES):
        m = dict(shared)
        m["tok"] = np.ascontiguousarray(tokens[c])
        m["img_t"] = np.ascontiguousarray(
            img[c].T.reshape(DIT, P, NI).transpose(1, 0, 2)).astype(BF16)
        in_maps.append(m)
    return in_maps


def _ensure_axon_hooks():
    """bass_utils imports antenv.axon_hooks when BASS_TRACE is set; stub it
    if the module is absent so tracing degrades instead of crashing."""
    try:
        import antenv.axon_hooks  # noqa: F401
    except ImportError:
        import types
        mod = types.ModuleType("antenv.axon_hooks")
        mod.get_axon_ntff_profile_hook = lambda: None
        mod.set_axon_ntff_profile_hook = lambda h: None
        sys.modules["antenv.axon_hooks"] = mod


def kernel(**inputs):
    global LAST_RESULTS
    _ensure_axon_hooks()
    from concourse.bass_utils import run_bass_kernel_spmd

    if "nc" not in _CACHE:
        _CACHE["nc"] = _build_program()
    nc = _CACHE["nc"]

    in_maps = _prep_inputs(inputs)
    res = run_bass_kernel_spmd(nc, in_maps, core_ids=list(range(N_CORES)))
    LAST_RESULTS = res
    out = np.stack([res.results[c]["out"].astype(np.float32)
                    for c in range(N_CORES)])
    return out
